# revision 13
# baseline (speedup 1.0000x reference)
"""HRR binding self-attention kernel for 8 trn2 NeuronCores (v2).

Math: out = irfft(c * rfft(x) * cumsum_s(rfft(x))) @ w_out.T  with c = queries*keyvalues.
rfft is linear, so cumsum commutes with it: one forward DFT of x; the causal
prefix sum runs in the frequency domain.  irfft is also linear, so it FUSES into
the output Linear: out = qv^T @ GW with GW = (c * Gf) @ w_out.T precomputed on
host (the c filter rides along for free since complex scalars commute).

Sharding: 8 shards = (batch b in 0..3) x (seq half h in 0..1), 2048 tokens each.
The h=1 shards get the first half's contribution as an initial carry, computed
on host as rfft(x[b, :2048].sum(0)) (negligible).

Packed real spectrum (2048 rows): rows 0..1024 = Re[0..1024], rows 1025..2047 =
Im[1..1023].  Row 1024 (Nyquist) rides in the Im-block's first slot (chunk 8,
partition 0); complex multiplies pair chunk c with chunk 8+c on equal
partitions, with a 2-row fixup for the DC/Nyquist slots.

Per-core pipeline, one pass over 8 slabs of 256 tokens (matmuls bf16, f32 PSUM):
  - transposed DFT: CS chunk stationary, x-slab moving -> freq-major spectrum
    [pk, tok] straight into PSUM (no token-major intermediate, no transpose);
  - Q copied to SBUF (ACT), then tensor_tensor_scan runs the causal cumsum
    in-place in PSUM (f32 state, per-partition carry chained across slabs);
  - complex multiply per chunk-pair (c, 8+c) on DVE -> qv bf16;
  - output matmul qv^T (stationary) @ GW (moving) -> out rows, f32.
Emission interleaves slab s's DFT with slab s-1's output matmul so the PE
never idles.
"""

import sys

sys.path.insert(0, "/opt/trn_rl_repo")

import numpy as np
import ml_dtypes

import concourse.bass as bass
import concourse.bacc as bacc
import concourse.mybir as mybir
from concourse.tile import TileContext
from concourse.bass_utils import run_bass_kernel_spmd

BF16 = mybir.dt.bfloat16
F32 = mybir.dt.float32
ADD = mybir.AluOpType.add
BYP = mybir.AluOpType.bypass

P = 128
D = 2048  # model dims
T = 2048  # tokens per shard
ND = D // P  # 16 d-chunks
NPF = 16  # packed-frequency chunks
TSB = 512  # tokens per slab
NSLAB = T // TSB  # 4
NB = 4  # batch
NS = 4096  # full seq

bf16 = ml_dtypes.bfloat16

_CACHE = {}


def _build_nc(reps: int = 1):
    nc = bacc.Bacc("TRN2", target_bir_lowering=False, debug=False, num_devices=8)
    xT = nc.dram_tensor("xT", [NSLAB, P, ND, TSB], BF16, kind="ExternalInput")
    CS2 = nc.dram_tensor("CS2", [NPF, P, ND, P], BF16, kind="ExternalInput")
    GW = nc.dram_tensor("GW", [P, NPF, D], BF16, kind="ExternalInput")
    C0 = nc.dram_tensor("C0", [P, NPF], F32, kind="ExternalInput")
    out = nc.dram_tensor("out", [T, D], BF16, kind="ExternalOutput")

    with TileContext(nc) as tc:
        with tc.tile_pool(name="misc", bufs=1) as misc:
            c0_sb = misc.tile([P, NPF], F32)
            nc.sync.dma_start(c0_sb[:], C0[:])

            import contextlib

            loop_ctx = (
                tc.For_i(0, reps, 1, staggered_reset=True)
                if reps > 1
                else contextlib.nullcontext()
            )
            with loop_ctx:
                _body(nc, tc, c0_sb, CS2, GW, xT, out)
    nc.finalize()
    return nc


def _body(nc, tc, c0_sb, CS2, GW, xT, out):
    with (
        tc.tile_pool(name="wts", bufs=1) as wpool,
        tc.tile_pool(name="xt", bufs=2) as xpool,
        tc.tile_pool(name="qsb", bufs=3) as qpool,
        tc.tile_pool(name="qv", bufs=2) as qvpool,
        tc.tile_pool(name="carry", bufs=2) as cpool,
        tc.tile_pool(name="tmp", bufs=1) as tpool,
        tc.tile_pool(name="osb", bufs=4) as opool,
        tc.tile_pool(name="psD", bufs=6, space="PSUM") as psD,
        tc.tile_pool(name="psC", bufs=2, space="PSUM") as psC,
    ):
        cs_sb = wpool.tile([P, NPF, ND, P], BF16)
        for pf in range(NPF):
            nc.sync.dma_start(cs_sb[:, pf], CS2[pf])
        gw_sb = wpool.tile([P, NPF, D], BF16)
        for pf in range(NPF):
            nc.sync.dma_start(gw_sb[:, pf, :], GW[:, pf, :])

        carry_prev = None
        qv_prev = None
        for s in range(NSLAB + 1):
            if s < NSLAB:
                xt = xpool.tile([P, ND, TSB], BF16, tag="xt")
                for q in range(4):
                    nc.sync.dma_start(xt[:, 4 * q : 4 * q + 4, :], xT[s, :, 4 * q : 4 * q + 4, :])
                qv = qvpool.tile([P, NPF, TSB], BF16, tag="qv")
                carry_sb = cpool.tile([P, NPF], F32, tag="carry")
                Qp0 = None
                for c in range(8):
                    Qp = qpool.tile([P, 2, TSB], BF16, tag="Q")
                    if c == 0:
                        Qp0 = Qp
                    psts = {}
                    for h, pf in enumerate((c, 8 + c)):
                        pst = psD.tile([P, TSB], F32, tag="psD")
                        for dc in range(ND):
                            nc.tensor.matmul(
                                pst[:],
                                cs_sb[:, pf, dc, :],
                                xt[:, dc, :],
                                start=(dc == 0),
                                stop=(dc == ND - 1),
                            )
                        nc.scalar.copy(Qp[:, h, :], pst[:])
                        init = (
                            c0_sb[:, pf : pf + 1]
                            if s == 0
                            else carry_prev[:, pf : pf + 1]
                        )
                        # op1=bypass: state = data0 + state; data1 ignored
                        nc.vector.tensor_tensor_scan(
                            pst[:], pst[:], Qp[:, h, :], init, ADD, BYP
                        )
                        nc.scalar.copy(carry_sb[:, pf : pf + 1], pst[:, TSB - 1 : TSB])
                        psts[h] = pst
                    SR, SI = psts[0], psts[1]
                    QR, QI = Qp[:, 0, :], Qp[:, 1, :]
                    t1 = tpool.tile([P, TSB], F32, tag="t1")
                    t2 = tpool.tile([P, TSB], F32, tag="t2")
                    nc.vector.tensor_mul(t1[:], QR, SR[:])
                    nc.vector.tensor_mul(t2[:], QI, SI[:])
                    nc.vector.tensor_sub(qv[:, c, :], t1[:], t2[:])
                    t3 = tpool.tile([P, TSB], F32, tag="t1")
                    t4 = tpool.tile([P, TSB], F32, tag="t2")
                    nc.vector.tensor_mul(t3[:], QR, SI[:])
                    nc.vector.tensor_mul(t4[:], QI, SR[:])
                    nc.vector.tensor_add(qv[:, 8 + c, :], t3[:], t4[:])
                    if c == 0:
                        # DC (chunk 0 row 0) and Nyquist (chunk 8 row 0): purely real
                        nc.vector.tensor_mul(qv[0:1, 0, :], Qp0[0:1, 0, :], SR[0:1, :])
                        nc.vector.tensor_mul(qv[0:1, 8, :], Qp0[0:1, 1, :], SI[0:1, :])
                carry_prev = carry_sb

            if s > 0:
                for tb in range(TSB // P):
                    for e in range(4):
                        psc = psC.tile([P, 512], F32, tag="psC")
                        for pf in range(NPF):
                            nc.tensor.matmul(
                                psc[:],
                                qv_prev[:, pf, tb * P : (tb + 1) * P],
                                gw_sb[:, pf, e * 512 : (e + 1) * 512],
                                start=(pf == 0),
                                stop=(pf == NPF - 1),
                            )
                        osb = opool.tile([P, 512], BF16, tag="osb")
                        if e % 2 == 0:
                            nc.scalar.copy(osb[:], psc[:])
                        else:
                            nc.vector.tensor_copy(osb[:], psc[:])
                        r0 = (s - 1) * TSB + tb * P
                        nc.sync.dma_start(
                            out[r0 : r0 + P, e * 512 : (e + 1) * 512], osb[:]
                        )
            if s < NSLAB:
                qv_prev = qv


def _chunked(m):
    """[rows, cols] -> [P, rows//P, cols] with row r at [r % P, r // P]."""
    r, c = m.shape
    return np.ascontiguousarray(m.reshape(r // P, P, c).transpose(1, 0, 2))


def _pack_spec(re, im):
    """re[1025], im[1025] -> packed [2048]: re[0..1024] then im[1..1023]."""
    return np.concatenate([re, im[1:1024]])


def _constants():
    if "consts" in _CACHE:
        return _CACHE["consts"]
    d = np.arange(D, dtype=np.float64)
    f = np.arange(D // 2 + 1, dtype=np.float64)
    ang = 2.0 * np.pi / D * np.outer(d, f)  # [D, 1025]
    cos, sin = np.cos(ang), np.sin(ang)
    CSf = np.concatenate([cos, -sin[:, 1:1024]], axis=1)  # [D, 2048] packed fwd
    alpha = np.full(1025, 2.0)
    alpha[0] = alpha[1024] = 1.0
    Gf = np.concatenate(
        [(alpha[:, None] * cos.T) / D, (-2.0 * sin[:, 1:1024].T) / D], axis=0
    )  # [2048 packed, D]
    # CS2[pf, p, dc, j] = CSf[128*dc + p, 128*pf + j]
    CS2 = np.ascontiguousarray(
        CSf.reshape(ND, P, NPF, P).transpose(2, 1, 0, 3)
    ).astype(np.float32)
    consts = {"CS2": CS2.astype(bf16), "Gf": Gf}
    _CACHE["consts"] = consts
    return consts


def kernel(x, queries, keyvalues, w_out):
    x = np.asarray(x, dtype=np.float32)
    queries = np.asarray(queries, dtype=np.float32)
    keyvalues = np.asarray(keyvalues, dtype=np.float32)
    w_out = np.asarray(w_out, dtype=np.float32)

    if "nc" not in _CACHE:
        _CACHE["nc"] = _build_nc()
    nc = _CACHE["nc"]
    consts = _constants()

    c = (queries * keyvalues).reshape(-1)  # [1025]
    c_packed = _pack_spec(c, c)  # [2048]
    GWf = (c_packed[:, None] * consts["Gf"]).astype(np.float32) @ w_out.T
    GWc = _chunked(GWf.astype(np.float32)).astype(bf16)  # [P, NPF, D]

    in_maps = []
    shards = []
    for b in range(NB):
        for h in range(2):
            shards.append((b, h))
            xs = x[b, h * T : (h + 1) * T]  # [T, D]
            xT3 = _chunked(np.ascontiguousarray(xs.T))  # [P, ND, T]
            xTc = np.ascontiguousarray(
                xT3.reshape(P, ND, NSLAB, TSB).transpose(2, 0, 1, 3)
            ).astype(bf16)
            if h == 0:
                c0 = np.zeros((P, NPF), np.float32)
            else:
                F = np.fft.rfft(x[b, :T].sum(axis=0).astype(np.float64))
                c0 = _chunked(
                    _pack_spec(F.real, F.imag).astype(np.float32)[:, None]
                )[:, :, 0]
            in_maps.append(
                {
                    "xT": xTc,
                    "CS2": consts["CS2"],
                    "GW": GWc,
                    "C0": np.ascontiguousarray(c0),
                }
            )

    global _LAST_IN_MAPS
    _LAST_IN_MAPS = in_maps
    res = run_bass_kernel_spmd(nc, in_maps, core_ids=list(range(8)))
    y = np.empty((NB, NS, D), np.float32)
    for i, (b, h) in enumerate(shards):
        y[b, h * T : (h + 1) * T] = res.results[i]["out"].astype(np.float32)
    return y


# revision 17
# speedup vs baseline: 1.5461x; 1.5461x over previous
"""HRR binding self-attention kernel for 8 trn2 NeuronCores (v2).

Math: out = irfft(c * rfft(x) * cumsum_s(rfft(x))) @ w_out.T  with c = queries*keyvalues.
rfft is linear, so cumsum commutes with it: one forward DFT of x; the causal
prefix sum runs in the frequency domain.  irfft is also linear, so it FUSES into
the output Linear: out = qv^T @ GW with GW = (c * Gf) @ w_out.T precomputed on
host (the c filter rides along for free since complex scalars commute).

Sharding: 8 shards = (batch b in 0..3) x (seq half h in 0..1), 2048 tokens each.
The h=1 shards get the first half's contribution as an initial carry, computed
on host as rfft(x[b, :2048].sum(0)) (negligible).

Packed real spectrum (2048 rows): rows 0..1024 = Re[0..1024], rows 1025..2047 =
Im[1..1023].  Row 1024 (Nyquist) rides in the Im-block's first slot (chunk 8,
partition 0); complex multiplies pair chunk c with chunk 8+c on equal
partitions, with a 2-row fixup for the DC/Nyquist slots.

Per-core pipeline, one pass over 8 slabs of 256 tokens (matmuls bf16, f32 PSUM):
  - transposed DFT: CS chunk stationary, x-slab moving -> freq-major spectrum
    [pk, tok] straight into PSUM (no token-major intermediate, no transpose);
  - Q copied to SBUF (ACT), then tensor_tensor_scan runs the causal cumsum
    in-place in PSUM (f32 state, per-partition carry chained across slabs);
  - complex multiply per chunk-pair (c, 8+c) on DVE -> qv bf16;
  - output matmul qv^T (stationary) @ GW (moving) -> out rows, f32.
Emission interleaves slab s's DFT with slab s-1's output matmul so the PE
never idles.
"""

import sys

sys.path.insert(0, "/opt/trn_rl_repo")

import numpy as np
import ml_dtypes

import concourse.bass as bass
import concourse.bacc as bacc
import concourse.mybir as mybir
from concourse.tile import TileContext
from concourse.bass_utils import run_bass_kernel_spmd

BF16 = mybir.dt.bfloat16
F32 = mybir.dt.float32
ADD = mybir.AluOpType.add
BYP = mybir.AluOpType.bypass

P = 128
D = 2048  # model dims
T = 2048  # tokens per shard
ND = D // P  # 16 d-chunks
NPF = 16  # packed-frequency chunks
TSB = 512  # tokens per slab
NSLAB = T // TSB  # 4
NB = 4  # batch
NS = 4096  # full seq

bf16 = ml_dtypes.bfloat16

_CACHE = {}


def _build_nc(reps: int = 1):
    nc = bacc.Bacc("TRN2", target_bir_lowering=False, debug=False, num_devices=8)
    xT = nc.dram_tensor("xT", [NSLAB, P, ND, TSB], BF16, kind="ExternalInput")
    CSE = nc.dram_tensor("CSE", [8, P, 8, P], BF16, kind="ExternalInput")
    CSO = nc.dram_tensor("CSO", [8, P, 8, P], BF16, kind="ExternalInput")
    GW = nc.dram_tensor("GW", [P, NPF, D], BF16, kind="ExternalInput")
    C0 = nc.dram_tensor("C0", [P, NPF], F32, kind="ExternalInput")
    out = nc.dram_tensor("out", [T, D], BF16, kind="ExternalOutput")

    with TileContext(nc) as tc:
        with tc.tile_pool(name="misc", bufs=1) as misc:
            c0_sb = misc.tile([P, NPF], F32)
            nc.sync.dma_start(c0_sb[:], C0[:])

            import contextlib

            loop_ctx = (
                tc.For_i(0, reps, 1, staggered_reset=True)
                if reps > 1
                else contextlib.nullcontext()
            )
            with loop_ctx:
                _body(nc, tc, c0_sb, CSE, CSO, GW, xT, out)
    nc.finalize()
    return nc


PAIRS = [(0, 4), (1, 5), (2, 6), (3, 7), (8, 12), (9, 13), (10, 14), (11, 15)]


def _body(nc, tc, c0_sb, CSE, CSO, GW, xT, out):
    with (
        tc.tile_pool(name="wts", bufs=1) as wpool,
        tc.tile_pool(name="xt", bufs=2) as xpool,
        tc.tile_pool(name="us", bufs=2) as uspool,
        tc.tile_pool(name="qsb", bufs=3) as qpool,
        tc.tile_pool(name="qv", bufs=2) as qvpool,
        tc.tile_pool(name="carry", bufs=2) as cpool,
        tc.tile_pool(name="tmp", bufs=1) as tpool,
        tc.tile_pool(name="osb", bufs=4) as opool,
        tc.tile_pool(name="psD", bufs=6, space="PSUM") as psD,
        tc.tile_pool(name="psC", bufs=2, space="PSUM") as psC,
    ):
        cse_sb = wpool.tile([P, 8, 8, P], BF16)
        for pf in range(8):
            nc.sync.dma_start(cse_sb[:, pf], CSE[pf])
        cso_sb = wpool.tile([P, 8, 8, P], BF16)
        for pf in range(8):
            nc.sync.dma_start(cso_sb[:, pf], CSO[pf])
        gw_sb = wpool.tile([P, NPF, D], BF16)
        for pf in range(NPF):
            nc.sync.dma_start(gw_sb[:, pf, :], GW[:, pf, :])

        carry_prev = None
        qv_prev = None
        for s in range(NSLAB + 1):
            if s < NSLAB:
                xt = xpool.tile([P, ND, TSB], BF16, tag="xt")
                for q in range(4):
                    nc.sync.dma_start(xt[:, 4 * q : 4 * q + 4, :], xT[s, :, 4 * q : 4 * q + 4, :])
                # u = x1 + x2 (chunks 0..7), sdiff = x1 - x2 (chunks 8..15)
                us = uspool.tile([P, NPF, TSB], BF16, tag="us")
                nc.vector.tensor_add(us[:, 0:8, :], xt[:, 0:8, :], xt[:, 8:16, :])
                nc.vector.tensor_sub(us[:, 8:16, :], xt[:, 0:8, :], xt[:, 8:16, :])
                qv = qvpool.tile([P, NPF, TSB], BF16, tag="qv")
                carry_sb = cpool.tile([P, NPF], F32, tag="carry")
                Qp0 = None
                for ci, (pfa, pfb) in enumerate(PAIRS):
                    Qp = qpool.tile([P, 2, TSB], BF16, tag="Q")
                    if ci == 0:
                        Qp0 = Qp
                    psts = {}
                    for h, pf in enumerate((pfa, pfb)):
                        ob = 0 if pf < 8 else 8  # E side reads u, O side reads sdiff
                        cs_sb = cse_sb if pf < 8 else cso_sb
                        pst = psD.tile([P, TSB], F32, tag="psD")
                        for dc in range(8):
                            nc.tensor.matmul(
                                pst[:],
                                cs_sb[:, pf % 8, dc, :],
                                us[:, ob + dc, :],
                                start=(dc == 0),
                                stop=(dc == 7),
                            )
                        nc.scalar.copy(Qp[:, h, :], pst[:])
                        init = (
                            c0_sb[:, pf : pf + 1]
                            if s == 0
                            else carry_prev[:, pf : pf + 1]
                        )
                        # op1=bypass: state = data0 + state; data1 ignored
                        nc.vector.tensor_tensor_scan(
                            pst[:], pst[:], Qp[:, h, :], init, ADD, BYP
                        )
                        nc.scalar.copy(carry_sb[:, pf : pf + 1], pst[:, TSB - 1 : TSB])
                        psts[h] = pst
                    SR, SI = psts[0], psts[1]
                    QR, QI = Qp[:, 0, :], Qp[:, 1, :]
                    t1 = tpool.tile([P, TSB], F32, tag="t1")
                    t2 = tpool.tile([P, TSB], F32, tag="t2")
                    nc.vector.tensor_mul(t1[:], QR, SR[:])
                    nc.vector.tensor_mul(t2[:], QI, SI[:])
                    nc.vector.tensor_sub(qv[:, pfa, :], t1[:], t2[:])
                    t3 = tpool.tile([P, TSB], F32, tag="t1")
                    t4 = tpool.tile([P, TSB], F32, tag="t2")
                    nc.vector.tensor_mul(t3[:], QR, SI[:])
                    nc.vector.tensor_mul(t4[:], QI, SR[:])
                    nc.vector.tensor_add(qv[:, pfb, :], t3[:], t4[:])
                    if ci == 0:
                        # DC (chunk 0 row 0) and Nyquist (chunk 4 row 0): purely real
                        nc.vector.tensor_mul(qv[0:1, 0, :], Qp0[0:1, 0, :], SR[0:1, :])
                        nc.vector.tensor_mul(qv[0:1, 4, :], Qp0[0:1, 1, :], SI[0:1, :])
                carry_prev = carry_sb

            if s > 0:
                for tb in range(TSB // P):
                    for e in range(4):
                        psc = psC.tile([P, 512], F32, tag="psC")
                        for pf in range(NPF):
                            nc.tensor.matmul(
                                psc[:],
                                qv_prev[:, pf, tb * P : (tb + 1) * P],
                                gw_sb[:, pf, e * 512 : (e + 1) * 512],
                                start=(pf == 0),
                                stop=(pf == NPF - 1),
                            )
                        osb = opool.tile([P, 512], BF16, tag="osb")
                        if e % 2 == 0:
                            nc.scalar.copy(osb[:], psc[:])
                        else:
                            nc.vector.tensor_copy(osb[:], psc[:])
                        r0 = (s - 1) * TSB + tb * P
                        nc.sync.dma_start(
                            out[r0 : r0 + P, e * 512 : (e + 1) * 512], osb[:]
                        )
            if s < NSLAB:
                qv_prev = qv


def _chunked(m):
    """[rows, cols] -> [P, rows//P, cols] with row r at [r % P, r // P]."""
    r, c = m.shape
    return np.ascontiguousarray(m.reshape(r // P, P, c).transpose(1, 0, 2))


def _pack_spec(re, im):
    """re[1025], im[1025] -> packed [2048]: re[0..1024] then im[1..1023]."""
    return np.concatenate([re, im[1:1024]])


def _constants():
    if "consts" in _CACHE:
        return _CACHE["consts"]
    H = D // 2
    d = np.arange(D, dtype=np.float64)
    f = np.arange(D // 2 + 1, dtype=np.float64)
    ang = 2.0 * np.pi / D * np.outer(d, f)  # [D, 1025]
    cos, sin = np.cos(ang), np.sin(ang)
    alpha = np.full(1025, 2.0)
    alpha[0] = alpha[1024] = 1.0
    Gf = np.concatenate(
        [(alpha[:, None] * cos.T) / D, (-2.0 * sin[:, 1:1024].T) / D], axis=0
    )  # [2048 std-packed, D]
    # DIF split: E = rfft_1024(x1+x2) covers even freqs; O = twiddle-folded
    # DFT_1024 of (x1-x2) covers odd freqs.
    d1 = np.arange(H, dtype=np.float64)
    mE = np.arange(H // 2 + 1, dtype=np.float64)
    angE = 2.0 * np.pi / H * np.outer(d1, mE)
    CS_E = np.concatenate([np.cos(angE), -np.sin(angE[:, 1:512])], axis=1)
    mO = np.arange(512, dtype=np.float64)
    thO = np.pi / H * np.outer(d1, 2 * mO + 1)
    CS_O = np.concatenate([np.cos(thO), -np.sin(thO)], axis=1)
    # chunked [pf, p, dc, j] = mat[128*dc + p, 128*pf + j]
    CSE2 = np.ascontiguousarray(CS_E.reshape(8, P, 8, P).transpose(2, 1, 0, 3))
    CSO2 = np.ascontiguousarray(CS_O.reshape(8, P, 8, P).transpose(2, 1, 0, 3))
    # new-basis row order: [E-pack 1024; O-pack 1024] -> std-packed row index
    perm = np.empty(2048, np.int64)
    perm[0:513] = 2 * np.arange(513)
    perm[513:1024] = 1024 + 2 * np.arange(1, 512)
    perm[1024:1536] = 2 * np.arange(512) + 1
    perm[1536:2048] = 1025 + 2 * np.arange(512)
    consts = {
        "CSE2": CSE2.astype(np.float32).astype(bf16),
        "CSO2": CSO2.astype(np.float32).astype(bf16),
        "Gf": Gf,
        "perm": perm,
    }
    _CACHE["consts"] = consts
    return consts


def kernel(x, queries, keyvalues, w_out):
    x = np.asarray(x, dtype=np.float32)
    queries = np.asarray(queries, dtype=np.float32)
    keyvalues = np.asarray(keyvalues, dtype=np.float32)
    w_out = np.asarray(w_out, dtype=np.float32)

    if "nc" not in _CACHE:
        _CACHE["nc"] = _build_nc()
    nc = _CACHE["nc"]
    consts = _constants()

    c = (queries * keyvalues).reshape(-1)  # [1025]
    c_packed = _pack_spec(c, c)  # [2048] std-packed
    GWf = (c_packed[:, None] * consts["Gf"]).astype(np.float32) @ w_out.T
    GWc = _chunked(GWf[consts["perm"]].astype(np.float32)).astype(bf16)

    in_maps = []
    shards = []
    for b in range(NB):
        for h in range(2):
            shards.append((b, h))
            xs = x[b, h * T : (h + 1) * T]  # [T, D]
            xT3 = _chunked(np.ascontiguousarray(xs.T))  # [P, ND, T]
            xTc = np.ascontiguousarray(
                xT3.reshape(P, ND, NSLAB, TSB).transpose(2, 0, 1, 3)
            ).astype(bf16)
            if h == 0:
                c0 = np.zeros((P, NPF), np.float32)
            else:
                F = np.fft.rfft(x[b, :T].sum(axis=0).astype(np.float64))
                c0s = _pack_spec(F.real, F.imag).astype(np.float32)
                c0 = _chunked(c0s[consts["perm"]][:, None])[:, :, 0]
            in_maps.append(
                {
                    "xT": xTc,
                    "CSE": consts["CSE2"],
                    "CSO": consts["CSO2"],
                    "GW": GWc,
                    "C0": np.ascontiguousarray(c0),
                }
            )

    global _LAST_IN_MAPS
    _LAST_IN_MAPS = in_maps
    res = run_bass_kernel_spmd(nc, in_maps, core_ids=list(range(8)))
    y = np.empty((NB, NS, D), np.float32)
    for i, (b, h) in enumerate(shards):
        y[b, h * T : (h + 1) * T] = res.results[i]["out"].astype(np.float32)
    return y


# revision 23
# speedup vs baseline: 1.5690x; 1.0148x over previous
"""HRR binding self-attention kernel for 8 trn2 NeuronCores (v2).

Math: out = irfft(c * rfft(x) * cumsum_s(rfft(x))) @ w_out.T  with c = queries*keyvalues.
rfft is linear, so cumsum commutes with it: one forward DFT of x; the causal
prefix sum runs in the frequency domain.  irfft is also linear, so it FUSES into
the output Linear: out = qv^T @ GW with GW = (c * Gf) @ w_out.T precomputed on
host (the c filter rides along for free since complex scalars commute).

Sharding: 8 shards = (batch b in 0..3) x (seq half h in 0..1), 2048 tokens each.
The h=1 shards get the first half's contribution as an initial carry, computed
on host as rfft(x[b, :2048].sum(0)) (negligible).

Packed real spectrum (2048 rows): rows 0..1024 = Re[0..1024], rows 1025..2047 =
Im[1..1023].  Row 1024 (Nyquist) rides in the Im-block's first slot (chunk 8,
partition 0); complex multiplies pair chunk c with chunk 8+c on equal
partitions, with a 2-row fixup for the DC/Nyquist slots.

Per-core pipeline, one pass over 8 slabs of 256 tokens (matmuls bf16, f32 PSUM):
  - transposed DFT: CS chunk stationary, x-slab moving -> freq-major spectrum
    [pk, tok] straight into PSUM (no token-major intermediate, no transpose);
  - Q copied to SBUF (ACT), then tensor_tensor_scan runs the causal cumsum
    in-place in PSUM (f32 state, per-partition carry chained across slabs);
  - complex multiply per chunk-pair (c, 8+c) on DVE -> qv bf16;
  - output matmul qv^T (stationary) @ GW (moving) -> out rows, f32.
Emission interleaves slab s's DFT with slab s-1's output matmul so the PE
never idles.
"""

import sys

sys.path.insert(0, "/opt/trn_rl_repo")

import numpy as np
import ml_dtypes

import concourse.bass as bass
import concourse.bacc as bacc
import concourse.mybir as mybir
from concourse.tile import TileContext
from concourse.bass_utils import run_bass_kernel_spmd

BF16 = mybir.dt.bfloat16
F32 = mybir.dt.float32
ADD = mybir.AluOpType.add
BYP = mybir.AluOpType.bypass

P = 128
D = 2048  # model dims
T = 2048  # tokens per shard
ND = D // P  # 16 d-chunks
NPF = 16  # packed-frequency chunks
TSB = 512  # tokens per slab
NSLAB = T // TSB  # 4
NB = 4  # batch
NS = 4096  # full seq

bf16 = ml_dtypes.bfloat16

_CACHE = {}


def _build_nc(reps: int = 1):
    nc = bacc.Bacc("TRN2", target_bir_lowering=False, debug=False, num_devices=8)
    xT = nc.dram_tensor("xT", [NSLAB, P, ND, TSB], BF16, kind="ExternalInput")
    CSEE = nc.dram_tensor("CSEE", [4, P, 4, P], BF16, kind="ExternalInput")
    CSEO = nc.dram_tensor("CSEO", [4, P, 4, P], BF16, kind="ExternalInput")
    CSO = nc.dram_tensor("CSO", [8, P, 8, P], BF16, kind="ExternalInput")
    GW = nc.dram_tensor("GW", [P, NPF, D], BF16, kind="ExternalInput")
    C0 = nc.dram_tensor("C0", [P, NPF], F32, kind="ExternalInput")
    out = nc.dram_tensor("out", [T, D], BF16, kind="ExternalOutput")

    with TileContext(nc) as tc:
        with tc.tile_pool(name="misc", bufs=1) as misc:
            c0_sb = misc.tile([P, NPF], F32)
            nc.sync.dma_start(c0_sb[:], C0[:])

            import contextlib

            loop_ctx = (
                tc.For_i(0, reps, 1, staggered_reset=True)
                if reps > 1
                else contextlib.nullcontext()
            )
            with loop_ctx:
                _body(nc, tc, c0_sb, CSEE, CSEO, CSO, GW, xT, out)
    nc.finalize()
    return nc


PAIRS = [(0, 2), (1, 3), (4, 6), (5, 7), (8, 12), (9, 13), (10, 14), (11, 15)]


def _body(nc, tc, c0_sb, CSEE, CSEO, CSO, GW, xT, out):
    with (
        tc.tile_pool(name="wts", bufs=1) as wpool,
        tc.tile_pool(name="xt", bufs=2) as xpool,
        tc.tile_pool(name="ut", bufs=1) as utpool,
        tc.tile_pool(name="us", bufs=2) as uspool,
        tc.tile_pool(name="qsb", bufs=3) as qpool,
        tc.tile_pool(name="qv", bufs=2) as qvpool,
        tc.tile_pool(name="carry", bufs=2) as cpool,
        tc.tile_pool(name="tmp", bufs=1) as tpool,
        tc.tile_pool(name="osb", bufs=4) as opool,
        tc.tile_pool(name="psD", bufs=6, space="PSUM") as psD,
        tc.tile_pool(name="psC", bufs=2, space="PSUM") as psC,
    ):
        csee_sb = wpool.tile([P, 4, 4, P], BF16)
        for pf in range(4):
            nc.sync.dma_start(csee_sb[:, pf], CSEE[pf])
        cseo_sb = wpool.tile([P, 4, 4, P], BF16)
        for pf in range(4):
            nc.sync.dma_start(cseo_sb[:, pf], CSEO[pf])
        cso_sb = wpool.tile([P, 8, 8, P], BF16)
        for pf in range(8):
            nc.sync.dma_start(cso_sb[:, pf], CSO[pf])
        gw_sb = wpool.tile([P, NPF, D], BF16)
        for pf in range(NPF):
            nc.sync.dma_start(gw_sb[:, pf, :], GW[:, pf, :])

        carry_prev = None
        qv_prev = None
        for s in range(NSLAB + 1):
            if s < NSLAB:
                xt = xpool.tile([P, ND, TSB], BF16, tag="xt")
                for q in range(4):
                    nc.sync.dma_start(xt[:, 4 * q : 4 * q + 4, :], xT[s, :, 4 * q : 4 * q + 4, :])
                # us chunks: 0..3 = uu = u1+u2, 4..7 = ud = u1-u2, 8..15 = x1-x2
                # where u = x1+x2, u1/u2 its halves
                u_t = utpool.tile([P, 8, TSB], BF16, tag="ut")
                nc.vector.tensor_add(u_t[:], xt[:, 0:8, :], xt[:, 8:16, :])
                us = uspool.tile([P, NPF, TSB], BF16, tag="us")
                nc.vector.tensor_sub(us[:, 8:16, :], xt[:, 0:8, :], xt[:, 8:16, :])
                nc.vector.tensor_add(us[:, 0:4, :], u_t[:, 0:4, :], u_t[:, 4:8, :])
                nc.vector.tensor_sub(us[:, 4:8, :], u_t[:, 0:4, :], u_t[:, 4:8, :])
                qv = qvpool.tile([P, NPF, TSB], BF16, tag="qv")
                carry_sb = cpool.tile([P, NPF], F32, tag="carry")
                Qp0 = None
                for ci, (pfa, pfb) in enumerate(PAIRS):
                    Qp = qpool.tile([P, 2, TSB], BF16, tag="Q")
                    if ci == 0:
                        Qp0 = Qp
                    psts = {}
                    for h, pf in enumerate((pfa, pfb)):
                        if pf < 4:  # EE: contracts uu
                            cs_ap, ob, ndc = csee_sb[:, pf], 0, 4
                        elif pf < 8:  # EO: contracts ud
                            cs_ap, ob, ndc = cseo_sb[:, pf - 4], 4, 4
                        else:  # O: contracts x1-x2
                            cs_ap, ob, ndc = cso_sb[:, pf - 8], 8, 8
                        pst = psD.tile([P, TSB], F32, tag="psD")
                        for dc in range(ndc):
                            nc.tensor.matmul(
                                pst[:],
                                cs_ap[:, dc, :],
                                us[:, ob + dc, :],
                                start=(dc == 0),
                                stop=(dc == ndc - 1),
                            )
                        nc.scalar.copy(Qp[:, h, :], pst[:])
                        init = (
                            c0_sb[:, pf : pf + 1]
                            if s == 0
                            else carry_prev[:, pf : pf + 1]
                        )
                        # op1=bypass: state = data0 + state; data1 ignored
                        nc.vector.tensor_tensor_scan(
                            pst[:], pst[:], Qp[:, h, :], init, ADD, BYP
                        )
                        nc.scalar.copy(carry_sb[:, pf : pf + 1], pst[:, TSB - 1 : TSB])
                        psts[h] = pst
                    SR, SI = psts[0], psts[1]
                    QR, QI = Qp[:, 0, :], Qp[:, 1, :]
                    t1 = tpool.tile([P, TSB], F32, tag="t1")
                    t2 = tpool.tile([P, TSB], F32, tag="t2")
                    nc.vector.tensor_mul(t1[:], QR, SR[:])
                    nc.vector.tensor_mul(t2[:], QI, SI[:])
                    nc.vector.tensor_sub(qv[:, pfa, :], t1[:], t2[:])
                    t3 = tpool.tile([P, TSB], F32, tag="t1")
                    t4 = tpool.tile([P, TSB], F32, tag="t2")
                    nc.vector.tensor_mul(t3[:], QR, SI[:])
                    nc.vector.tensor_mul(t4[:], QI, SR[:])
                    nc.vector.tensor_add(qv[:, pfb, :], t3[:], t4[:])
                    if ci == 0:
                        # DC (chunk 0 row 0) and Nyquist (chunk 2 row 0): purely real
                        nc.vector.tensor_mul(qv[0:1, 0, :], Qp0[0:1, 0, :], SR[0:1, :])
                        nc.vector.tensor_mul(qv[0:1, 2, :], Qp0[0:1, 1, :], SI[0:1, :])
                carry_prev = carry_sb

            if s > 0:
                for tb in range(TSB // P):
                    for e in range(4):
                        psc = psC.tile([P, 512], F32, tag="psC")
                        for pf in range(NPF):
                            nc.tensor.matmul(
                                psc[:],
                                qv_prev[:, pf, tb * P : (tb + 1) * P],
                                gw_sb[:, pf, e * 512 : (e + 1) * 512],
                                start=(pf == 0),
                                stop=(pf == NPF - 1),
                            )
                        osb = opool.tile([P, 512], BF16, tag="osb")
                        if e % 2 == 0:
                            nc.scalar.copy(osb[:], psc[:])
                        else:
                            nc.vector.tensor_copy(osb[:], psc[:])
                        r0 = (s - 1) * TSB + tb * P
                        nc.sync.dma_start(
                            out[r0 : r0 + P, e * 512 : (e + 1) * 512], osb[:]
                        )
            if s < NSLAB:
                qv_prev = qv


def _chunked(m):
    """[rows, cols] -> [P, rows//P, cols] with row r at [r % P, r // P]."""
    r, c = m.shape
    return np.ascontiguousarray(m.reshape(r // P, P, c).transpose(1, 0, 2))


def _pack_spec(re, im):
    """re[1025], im[1025] -> packed [2048]: re[0..1024] then im[1..1023]."""
    return np.concatenate([re, im[1:1024]])


def _constants():
    if "consts" in _CACHE:
        return _CACHE["consts"]
    H = D // 2
    d = np.arange(D, dtype=np.float64)
    f = np.arange(D // 2 + 1, dtype=np.float64)
    ang = 2.0 * np.pi / D * np.outer(d, f)  # [D, 1025]
    cos, sin = np.cos(ang), np.sin(ang)
    alpha = np.full(1025, 2.0)
    alpha[0] = alpha[1024] = 1.0
    Gf = np.concatenate(
        [(alpha[:, None] * cos.T) / D, (-2.0 * sin[:, 1:1024].T) / D], axis=0
    )  # [2048 std-packed, D]
    # Two-level DIF split: EE = rfft_512(uu) covers freqs 4m; EO =
    # twiddle-folded DFT_512(ud) covers 4m+2; O = twiddle-folded DFT_1024 of
    # (x1-x2) covers odd freqs. All twiddles live in the host matrices.
    Qd = D // 4
    d2 = np.arange(Qd, dtype=np.float64)
    mEE = np.arange(Qd // 2 + 1, dtype=np.float64)
    angEE = 2.0 * np.pi / Qd * np.outer(d2, mEE)
    CS_EE = np.concatenate([np.cos(angEE), -np.sin(angEE[:, 1:256])], axis=1)
    mEO = np.arange(256, dtype=np.float64)
    thEO = np.pi / Qd * np.outer(d2, 2 * mEO + 1)
    CS_EO = np.concatenate([np.cos(thEO), -np.sin(thEO)], axis=1)
    d1 = np.arange(H, dtype=np.float64)
    mO = np.arange(512, dtype=np.float64)
    thO = np.pi / H * np.outer(d1, 2 * mO + 1)
    CS_O = np.concatenate([np.cos(thO), -np.sin(thO)], axis=1)
    # chunked [pf, p, dc, j] = mat[128*dc + p, 128*pf + j]
    CSEE2 = np.ascontiguousarray(CS_EE.reshape(4, P, 4, P).transpose(2, 1, 0, 3))
    CSEO2 = np.ascontiguousarray(CS_EO.reshape(4, P, 4, P).transpose(2, 1, 0, 3))
    CSO2 = np.ascontiguousarray(CS_O.reshape(8, P, 8, P).transpose(2, 1, 0, 3))
    # new-basis row order: [EE 512; EO 512; O 1024] -> std-packed row index
    perm = np.empty(2048, np.int64)
    perm[0:257] = 4 * np.arange(257)
    perm[257:512] = 1024 + 4 * np.arange(1, 256)
    perm[512:768] = 4 * np.arange(256) + 2
    perm[768:1024] = 1024 + 4 * np.arange(256) + 2
    perm[1024:1536] = 2 * np.arange(512) + 1
    perm[1536:2048] = 1025 + 2 * np.arange(512)
    consts = {
        "CSEE2": CSEE2.astype(np.float32).astype(bf16),
        "CSEO2": CSEO2.astype(np.float32).astype(bf16),
        "CSO2": CSO2.astype(np.float32).astype(bf16),
        "Gf": Gf,
        "perm": perm,
    }
    _CACHE["consts"] = consts
    return consts


def kernel(x, queries, keyvalues, w_out):
    x = np.asarray(x, dtype=np.float32)
    queries = np.asarray(queries, dtype=np.float32)
    keyvalues = np.asarray(keyvalues, dtype=np.float32)
    w_out = np.asarray(w_out, dtype=np.float32)

    if "nc" not in _CACHE:
        _CACHE["nc"] = _build_nc()
    nc = _CACHE["nc"]
    consts = _constants()

    c = (queries * keyvalues).reshape(-1)  # [1025]
    c_packed = _pack_spec(c, c)  # [2048] std-packed
    GWf = (c_packed[:, None] * consts["Gf"]).astype(np.float32) @ w_out.T
    GWc = _chunked(GWf[consts["perm"]].astype(np.float32)).astype(bf16)

    in_maps = []
    shards = []
    for b in range(NB):
        for h in range(2):
            shards.append((b, h))
            xs = x[b, h * T : (h + 1) * T]  # [T, D]
            xT3 = _chunked(np.ascontiguousarray(xs.T))  # [P, ND, T]
            xTc = np.ascontiguousarray(
                xT3.reshape(P, ND, NSLAB, TSB).transpose(2, 0, 1, 3)
            ).astype(bf16)
            if h == 0:
                c0 = np.zeros((P, NPF), np.float32)
            else:
                F = np.fft.rfft(x[b, :T].sum(axis=0).astype(np.float64))
                c0s = _pack_spec(F.real, F.imag).astype(np.float32)
                c0 = _chunked(c0s[consts["perm"]][:, None])[:, :, 0]
            in_maps.append(
                {
                    "xT": xTc,
                    "CSEE": consts["CSEE2"],
                    "CSEO": consts["CSEO2"],
                    "CSO": consts["CSO2"],
                    "GW": GWc,
                    "C0": np.ascontiguousarray(c0),
                }
            )

    global _LAST_IN_MAPS
    _LAST_IN_MAPS = in_maps
    res = run_bass_kernel_spmd(nc, in_maps, core_ids=list(range(8)))
    y = np.empty((NB, NS, D), np.float32)
    for i, (b, h) in enumerate(shards):
        y[b, h * T : (h + 1) * T] = res.results[i]["out"].astype(np.float32)
    return y


# revision 24
# speedup vs baseline: 1.6601x; 1.0580x over previous
"""HRR binding self-attention kernel for 8 trn2 NeuronCores.

Math: out = irfft(c * rfft(x) * cumsum_s(rfft(x))) @ w_out.T, c = queries*keyvalues.
rfft is linear, so the causal cumsum commutes into the frequency domain; irfft
is linear, so it fuses into the output Linear: out = qv^T @ GW with
GW = (c * Gf) @ w_out.T precomputed on host (the real filter c commutes with
the complex products and rides along for free).

The forward DFT uses a two-level decimation-in-frequency split with every
twiddle folded into host-precomputed matrices (twiddles depend only on the
contraction index, so they fold; the even/odd output interleave is absorbed
into GW's row order):
  EE = rfft_512(uu)            covers freqs 4m        (uu = u1+u2, u = x1+x2)
  EO = DFT'_512(ud)            covers freqs 4m+2      (ud = u1-u2, twiddled)
  O  = DFT'_1024(x1-x2)        covers odd freqs       (twiddled)
This halves-then-quarters the DFT matmul count (256 -> 96 per slab); deeper
splits hit complex-input subbranches or output-set overlap (cross terms grow
the output contraction by exactly the DFT saving), so this is the fixed point.

Sharding: 8 shards = (batch b in 0..3) x (seq half h in 0..1), 2048 tokens
each.  h=1 shards get the first half's contribution as an initial carry,
computed on host as rfft(x[b, :2048].sum(0)) (negligible).

New-basis packed spectrum (2048 rows = 16 chunks of 128): [EE 512 | EO 512 |
O 1024], each block packed Re-then-Im so complex multiplies pair chunks on
equal partitions: pairs (0,2),(1,3) | (4,6),(5,7) | (8,12)..(11,15), with a
2-row fixup for the DC/Nyquist slots (chunk 0 row 0, chunk 2 row 0).

Per-core single pass over 4 slabs of 512 tokens (matmuls bf16, f32 PSUM):
  - u/s prep: 4 wide DVE adds/subs on the x-chunks;
  - transposed DFT: CS chunks stationary, u/s moving -> freq-major spectrum
    [pk, tok] straight into PSUM (no token-major intermediate, no transpose);
  - Q copied to SBUF (ACT), then tensor_tensor_scan runs the causal cumsum
    in-place in PSUM (f32 state, per-partition carry chained across slabs);
  - complex multiply per chunk-pair on DVE -> qv bf16;
  - output matmul qv (stationary) @ GW (moving) -> out rows, bf16 staging
    (host casts back to f32).
Emission interleaves slab s's DFT with slab s-1's output matmul so the PE
never idles; the reps loop uses For_i(staggered_reset=True) so iterations
overlap without an all-engine barrier.
"""

import sys

sys.path.insert(0, "/opt/trn_rl_repo")

import numpy as np
import ml_dtypes

import concourse.bass as bass
import concourse.bacc as bacc
import concourse.mybir as mybir
from concourse.tile import TileContext
from concourse.bass_utils import run_bass_kernel_spmd

BF16 = mybir.dt.bfloat16
F32 = mybir.dt.float32
ADD = mybir.AluOpType.add
BYP = mybir.AluOpType.bypass

P = 128
D = 2048  # model dims
T = 2048  # tokens per shard
ND = D // P  # 16 d-chunks
NPF = 16  # packed-frequency chunks
TSB = 512  # tokens per slab
NSLAB = T // TSB  # 4
NB = 4  # batch
NS = 4096  # full seq

bf16 = ml_dtypes.bfloat16

_CACHE = {}


def _build_nc(reps: int = 1):
    nc = bacc.Bacc("TRN2", target_bir_lowering=False, debug=False, num_devices=8)
    xT = nc.dram_tensor("xT", [NSLAB, P, ND, TSB], BF16, kind="ExternalInput")
    CSEE = nc.dram_tensor("CSEE", [4, P, 4, P], BF16, kind="ExternalInput")
    CSEO = nc.dram_tensor("CSEO", [4, P, 4, P], BF16, kind="ExternalInput")
    CSO = nc.dram_tensor("CSO", [8, P, 8, P], BF16, kind="ExternalInput")
    GW = nc.dram_tensor("GW", [P, NPF, D], BF16, kind="ExternalInput")
    C0 = nc.dram_tensor("C0", [P, NPF], F32, kind="ExternalInput")
    out = nc.dram_tensor("out", [T, D], BF16, kind="ExternalOutput")

    with TileContext(nc) as tc:
        with tc.tile_pool(name="misc", bufs=1) as misc:
            c0_sb = misc.tile([P, NPF], F32)
            nc.sync.dma_start(c0_sb[:], C0[:])

            import contextlib

            loop_ctx = (
                tc.For_i(0, reps, 1, staggered_reset=True)
                if reps > 1
                else contextlib.nullcontext()
            )
            with loop_ctx:
                _body(nc, tc, c0_sb, CSEE, CSEO, CSO, GW, xT, out)
    nc.finalize()
    return nc


PAIRS = [(0, 2), (1, 3), (4, 6), (5, 7), (8, 12), (9, 13), (10, 14), (11, 15)]


def _body(nc, tc, c0_sb, CSEE, CSEO, CSO, GW, xT, out):
    with (
        tc.tile_pool(name="wts", bufs=1) as wpool,
        tc.tile_pool(name="xt", bufs=2) as xpool,
        tc.tile_pool(name="ut", bufs=1) as utpool,
        tc.tile_pool(name="us", bufs=2) as uspool,
        tc.tile_pool(name="qsb", bufs=3) as qpool,
        tc.tile_pool(name="qv", bufs=2) as qvpool,
        tc.tile_pool(name="carry", bufs=2) as cpool,
        tc.tile_pool(name="tmp", bufs=1) as tpool,
        tc.tile_pool(name="osb", bufs=4) as opool,
        tc.tile_pool(name="psD", bufs=6, space="PSUM") as psD,
        tc.tile_pool(name="psC", bufs=2, space="PSUM") as psC,
    ):
        csee_sb = wpool.tile([P, 4, 4, P], BF16)
        for pf in range(4):
            nc.sync.dma_start(csee_sb[:, pf], CSEE[pf])
        cseo_sb = wpool.tile([P, 4, 4, P], BF16)
        for pf in range(4):
            nc.sync.dma_start(cseo_sb[:, pf], CSEO[pf])
        cso_sb = wpool.tile([P, 8, 8, P], BF16)
        for pf in range(8):
            nc.sync.dma_start(cso_sb[:, pf], CSO[pf])
        gw_sb = wpool.tile([P, NPF, D], BF16)
        for pf in range(NPF):
            nc.sync.dma_start(gw_sb[:, pf, :], GW[:, pf, :])

        carry_prev = None
        qv_prev = None
        for s in range(NSLAB + 1):
            if s < NSLAB:
                xt = xpool.tile([P, ND, TSB], BF16, tag="xt")
                for q in range(4):
                    nc.sync.dma_start(xt[:, 4 * q : 4 * q + 4, :], xT[s, :, 4 * q : 4 * q + 4, :])
                # us chunks: 0..3 = uu = u1+u2, 4..7 = ud = u1-u2, 8..15 = x1-x2
                # where u = x1+x2, u1/u2 its halves
                u_t = utpool.tile([P, 8, TSB], BF16, tag="ut")
                nc.vector.tensor_add(u_t[:], xt[:, 0:8, :], xt[:, 8:16, :])
                us = uspool.tile([P, NPF, TSB], BF16, tag="us")
                nc.vector.tensor_sub(us[:, 8:16, :], xt[:, 0:8, :], xt[:, 8:16, :])
                nc.vector.tensor_add(us[:, 0:4, :], u_t[:, 0:4, :], u_t[:, 4:8, :])
                nc.vector.tensor_sub(us[:, 4:8, :], u_t[:, 0:4, :], u_t[:, 4:8, :])
                qv = qvpool.tile([P, NPF, TSB], BF16, tag="qv")
                carry_sb = cpool.tile([P, NPF], F32, tag="carry")
                Qp0 = None
                for ci, (pfa, pfb) in enumerate(PAIRS):
                    Qp = qpool.tile([P, 2, TSB], BF16, tag="Q")
                    if ci == 0:
                        Qp0 = Qp
                    psts = {}
                    for h, pf in enumerate((pfa, pfb)):
                        if pf < 4:  # EE: contracts uu
                            cs_ap, ob, ndc = csee_sb[:, pf], 0, 4
                        elif pf < 8:  # EO: contracts ud
                            cs_ap, ob, ndc = cseo_sb[:, pf - 4], 4, 4
                        else:  # O: contracts x1-x2
                            cs_ap, ob, ndc = cso_sb[:, pf - 8], 8, 8
                        pst = psD.tile([P, TSB], F32, tag="psD")
                        for dc in range(ndc):
                            nc.tensor.matmul(
                                pst[:],
                                cs_ap[:, dc, :],
                                us[:, ob + dc, :],
                                start=(dc == 0),
                                stop=(dc == ndc - 1),
                            )
                        nc.scalar.copy(Qp[:, h, :], pst[:])
                        init = (
                            c0_sb[:, pf : pf + 1]
                            if s == 0
                            else carry_prev[:, pf : pf + 1]
                        )
                        # op1=bypass: state = data0 + state; data1 ignored
                        nc.vector.tensor_tensor_scan(
                            pst[:], pst[:], Qp[:, h, :], init, ADD, BYP
                        )
                        nc.scalar.copy(carry_sb[:, pf : pf + 1], pst[:, TSB - 1 : TSB])
                        psts[h] = pst
                    SR, SI = psts[0], psts[1]
                    QR, QI = Qp[:, 0, :], Qp[:, 1, :]
                    t1 = tpool.tile([P, TSB], F32, tag="t1")
                    t2 = tpool.tile([P, TSB], F32, tag="t2")
                    nc.vector.tensor_mul(t1[:], QR, SR[:])
                    nc.vector.tensor_mul(t2[:], QI, SI[:])
                    nc.vector.tensor_sub(qv[:, pfa, :], t1[:], t2[:])
                    t3 = tpool.tile([P, TSB], F32, tag="t1")
                    t4 = tpool.tile([P, TSB], F32, tag="t2")
                    nc.vector.tensor_mul(t3[:], QR, SI[:])
                    nc.vector.tensor_mul(t4[:], QI, SR[:])
                    nc.vector.tensor_add(qv[:, pfb, :], t3[:], t4[:])
                    if ci == 0:
                        # DC (chunk 0 row 0) and Nyquist (chunk 2 row 0): purely real
                        nc.vector.tensor_mul(qv[0:1, 0, :], Qp0[0:1, 0, :], SR[0:1, :])
                        nc.vector.tensor_mul(qv[0:1, 2, :], Qp0[0:1, 1, :], SI[0:1, :])
                carry_prev = carry_sb

            if s > 0:
                for tb in range(TSB // P):
                    for e in range(4):
                        psc = psC.tile([P, 512], F32, tag="psC")
                        for pf in range(NPF):
                            nc.tensor.matmul(
                                psc[:],
                                qv_prev[:, pf, tb * P : (tb + 1) * P],
                                gw_sb[:, pf, e * 512 : (e + 1) * 512],
                                start=(pf == 0),
                                stop=(pf == NPF - 1),
                            )
                        osb = opool.tile([P, 512], BF16, tag="osb")
                        if e % 2 == 0:
                            nc.scalar.copy(osb[:], psc[:])
                        else:
                            nc.vector.tensor_copy(osb[:], psc[:])
                        r0 = (s - 1) * TSB + tb * P
                        nc.sync.dma_start(
                            out[r0 : r0 + P, e * 512 : (e + 1) * 512], osb[:]
                        )
            if s < NSLAB:
                qv_prev = qv


def _chunked(m):
    """[rows, cols] -> [P, rows//P, cols] with row r at [r % P, r // P]."""
    r, c = m.shape
    return np.ascontiguousarray(m.reshape(r // P, P, c).transpose(1, 0, 2))


def _pack_spec(re, im):
    """re[1025], im[1025] -> packed [2048]: re[0..1024] then im[1..1023]."""
    return np.concatenate([re, im[1:1024]])


def _constants():
    if "consts" in _CACHE:
        return _CACHE["consts"]
    H = D // 2
    d = np.arange(D, dtype=np.float64)
    f = np.arange(D // 2 + 1, dtype=np.float64)
    ang = 2.0 * np.pi / D * np.outer(d, f)  # [D, 1025]
    cos, sin = np.cos(ang), np.sin(ang)
    alpha = np.full(1025, 2.0)
    alpha[0] = alpha[1024] = 1.0
    Gf = np.concatenate(
        [(alpha[:, None] * cos.T) / D, (-2.0 * sin[:, 1:1024].T) / D], axis=0
    )  # [2048 std-packed, D]
    # Two-level DIF split: EE = rfft_512(uu) covers freqs 4m; EO =
    # twiddle-folded DFT_512(ud) covers 4m+2; O = twiddle-folded DFT_1024 of
    # (x1-x2) covers odd freqs. All twiddles live in the host matrices.
    Qd = D // 4
    d2 = np.arange(Qd, dtype=np.float64)
    mEE = np.arange(Qd // 2 + 1, dtype=np.float64)
    angEE = 2.0 * np.pi / Qd * np.outer(d2, mEE)
    CS_EE = np.concatenate([np.cos(angEE), -np.sin(angEE[:, 1:256])], axis=1)
    mEO = np.arange(256, dtype=np.float64)
    thEO = np.pi / Qd * np.outer(d2, 2 * mEO + 1)
    CS_EO = np.concatenate([np.cos(thEO), -np.sin(thEO)], axis=1)
    d1 = np.arange(H, dtype=np.float64)
    mO = np.arange(512, dtype=np.float64)
    thO = np.pi / H * np.outer(d1, 2 * mO + 1)
    CS_O = np.concatenate([np.cos(thO), -np.sin(thO)], axis=1)
    # chunked [pf, p, dc, j] = mat[128*dc + p, 128*pf + j]
    CSEE2 = np.ascontiguousarray(CS_EE.reshape(4, P, 4, P).transpose(2, 1, 0, 3))
    CSEO2 = np.ascontiguousarray(CS_EO.reshape(4, P, 4, P).transpose(2, 1, 0, 3))
    CSO2 = np.ascontiguousarray(CS_O.reshape(8, P, 8, P).transpose(2, 1, 0, 3))
    # new-basis row order: [EE 512; EO 512; O 1024] -> std-packed row index
    perm = np.empty(2048, np.int64)
    perm[0:257] = 4 * np.arange(257)
    perm[257:512] = 1024 + 4 * np.arange(1, 256)
    perm[512:768] = 4 * np.arange(256) + 2
    perm[768:1024] = 1024 + 4 * np.arange(256) + 2
    perm[1024:1536] = 2 * np.arange(512) + 1
    perm[1536:2048] = 1025 + 2 * np.arange(512)
    consts = {
        "CSEE2": CSEE2.astype(np.float32).astype(bf16),
        "CSEO2": CSEO2.astype(np.float32).astype(bf16),
        "CSO2": CSO2.astype(np.float32).astype(bf16),
        "Gf": Gf,
        "perm": perm,
    }
    _CACHE["consts"] = consts
    return consts


def kernel(x, queries, keyvalues, w_out):
    x = np.asarray(x, dtype=np.float32)
    queries = np.asarray(queries, dtype=np.float32)
    keyvalues = np.asarray(keyvalues, dtype=np.float32)
    w_out = np.asarray(w_out, dtype=np.float32)

    if "nc" not in _CACHE:
        _CACHE["nc"] = _build_nc()
    nc = _CACHE["nc"]
    consts = _constants()

    c = (queries * keyvalues).reshape(-1)  # [1025]
    c_packed = _pack_spec(c, c)  # [2048] std-packed
    GWf = (c_packed[:, None] * consts["Gf"]).astype(np.float32) @ w_out.T
    GWc = _chunked(GWf[consts["perm"]].astype(np.float32)).astype(bf16)

    in_maps = []
    shards = []
    for b in range(NB):
        for h in range(2):
            shards.append((b, h))
            xs = x[b, h * T : (h + 1) * T]  # [T, D]
            xT3 = _chunked(np.ascontiguousarray(xs.T))  # [P, ND, T]
            xTc = np.ascontiguousarray(
                xT3.reshape(P, ND, NSLAB, TSB).transpose(2, 0, 1, 3)
            ).astype(bf16)
            if h == 0:
                c0 = np.zeros((P, NPF), np.float32)
            else:
                F = np.fft.rfft(x[b, :T].sum(axis=0).astype(np.float64))
                c0s = _pack_spec(F.real, F.imag).astype(np.float32)
                c0 = _chunked(c0s[consts["perm"]][:, None])[:, :, 0]
            in_maps.append(
                {
                    "xT": xTc,
                    "CSEE": consts["CSEE2"],
                    "CSEO": consts["CSEO2"],
                    "CSO": consts["CSO2"],
                    "GW": GWc,
                    "C0": np.ascontiguousarray(c0),
                }
            )

    global _LAST_IN_MAPS
    _LAST_IN_MAPS = in_maps
    res = run_bass_kernel_spmd(nc, in_maps, core_ids=list(range(8)))
    y = np.empty((NB, NS, D), np.float32)
    for i, (b, h) in enumerate(shards):
        y[b, h * T : (h + 1) * T] = res.results[i]["out"].astype(np.float32)
    return y
